# revision 28
# baseline (speedup 1.0000x reference)
"""Trainium2 Bass kernel for nn_Decoder_8950711845590 (PointNet++-style decoder).

Self-contained: builds, compiles and runs an 8-core SPMD Bass/Tile kernel.
Sharding: data-parallel over point clouds (2 clouds per core), MLP weights
replicated; BatchNorm batch-statistics are combined across cores with a tiny
AllReduce per BN layer.

Algorithm highlights (per core):
- KNN distances via PE matmul with K=4 ("[qx,qy,qz,-1] . [2px,2py,2pz,|p|^2]").
- top-k via one DVE max8 pass over values packed as fp32 = [bf16(-d) | idx]
  (value-sorted with index tie-break, index extracted from the low 16 bits).
- k=3 stages screen top-4 in bf16, then re-compute exact fp32 distances for
  the 4 survivors from gathered candidate coordinates and drop the farthest.
- feature gather through indirect DMA from DRAM row tables [feat | pos | p^2].
- weighted combine with PE matmuls against per-query diagonal weight matrices.
- feature-major MLPs (channels on partitions) so BN stats are free-dim reduces.
"""

import os
import numpy as np

import concourse.bass as bass
import concourse.mybir as mybir
import concourse.tile as tile
from concourse import bacc
from concourse.bass import ts, ds, IndirectOffsetOnAxis
from concourse.masks import make_identity

AF = mybir.ActivationFunctionType
ALU = mybir.AluOpType
DT = mybir.dt
F32 = DT.float32
F32R = DT.float32r
BF16 = DT.bfloat16
FP16 = DT.float16
U16 = DT.uint16
U32 = DT.uint32
I32 = DT.int32

P = 128
EPS_BN = 1e-5

# problem constants (full config)
B_FULL = 16
NPT_FULL = (4096, 1024, 256, 64)
NCORES_FULL = 8
CPC = 2  # clouds per core

# feature dims (fixed by the MLP weights)
C3F = 1024   # x3 feature dim
FP3_IN, FP3_HID, FP3_OUT = 1280, 256, 256
FP2_IN, FP2_HID, FP2_OUT = 384, 256, 128
FP1_IN, FP1_HID, FP1_OUT = 131, 128, 128
HEAD_HID, HEAD_OUT = 128, 3

KS = 4  # screened survivors per query for k=3 stages


def _chunks(n, c):
    out = []
    o = 0
    while o < n:
        w = min(c, n - o)
        out.append((o, w))
        o += w
    return out


class Cfg:
    def __init__(self, npt=NPT_FULL, cpc=CPC, ncores=NCORES_FULL):
        self.npt = npt
        self.cpc = cpc
        self.ncores = ncores
        self.n0, self.n1, self.n2, self.n3 = [cpc * n for n in npt]
        self.c1, self.c2, self.c3 = npt[1], npt[2], npt[3]
        assert self.n0 % P == 0 and npt[0] % P == 0
        assert npt[1] % P == 0 and npt[2] % P == 0
        assert self.c3 <= P and self.n3 <= P
        self.T0 = self.n0 // P          # fp1 query tiles
        self.T1 = self.n1 // P          # fp2 query tiles (and h1aug tiles)
        self.T2 = self.n2 // P          # fp3 query tiles (and h2aug tiles)
        self.t0c = npt[0] // P          # tiles per cloud
        self.t1c = npt[1] // P
        self.t2c = npt[2] // P
        self.TB1 = min(4, self.T0)      # gather batch (fp1), = 512-pt N chunk
        self.TB2 = min(2, self.T1)
        self.AUG1 = FP2_OUT + 4         # 132
        self.AUG2 = FP3_OUT + 4         # 260


def build(nc, cfg: Cfg):
    c = cfg
    do_ar = c.ncores > 1

    # ---------------- DRAM I/O ----------------
    def din(name, shape):
        return nc.dram_tensor(name, list(shape), F32, kind="ExternalInput")

    px0 = din("px0", (c.n0, 8))          # [pos0 | -1 | x0 | 0]
    pq1 = din("pq1", (c.n1, 4))          # [pos1 | -1]
    pq2 = din("pq2", (c.n2, 4))          # [pos2 | -1]
    pos3 = din("pos3", (c.n3, 3))
    x1T = din("x1T", (FP2_OUT, c.n1))
    x2T = din("x2T", (FP3_OUT, c.n2))
    x3 = din("x3", (c.n3, C3F))
    w = {}
    for name, shape in [
        ("fp3_W1", (FP3_IN, FP3_HID)), ("fp3_g1", (FP3_HID,)), ("fp3_be1", (FP3_HID,)),
        ("fp3_W2", (FP3_HID, FP3_OUT)), ("fp3_b2", (FP3_OUT,)),
        ("fp2_W1", (FP2_IN, FP2_HID)), ("fp2_g1", (FP2_HID,)), ("fp2_be1", (FP2_HID,)),
        ("fp2_W2", (FP2_HID, FP2_OUT)), ("fp2_b2", (FP2_OUT,)),
        ("fp1_W1", (FP1_IN, FP1_HID)), ("fp1_g1", (FP1_HID,)), ("fp1_be1", (FP1_HID,)),
        ("fp1_W2", (FP1_HID, FP1_HID)), ("fp1_g2", (FP1_HID,)), ("fp1_be2", (FP1_HID,)),
        ("fp1_W3", (FP1_HID, FP1_OUT)), ("fp1_b3", (FP1_OUT,)),
        ("head_Wa", (FP1_OUT, HEAD_HID)), ("head_ba", (HEAD_HID,)),
        ("head_Wb", (HEAD_HID, HEAD_HID)), ("head_bb", (HEAD_HID,)),
        ("head_Wc", (HEAD_HID, HEAD_OUT)), ("head_bc", (HEAD_OUT,)),
    ]:
        w[name] = din(name, shape)
    outT = nc.dram_tensor("outT", [HEAD_OUT, c.n0], F32, kind="ExternalOutput")

    # DRAM scratch
    h1aug_d = nc.dram_tensor("h1aug", [c.n1, c.AUG1], F32, kind="Internal")
    h2aug_d = nc.dram_tensor("h2aug", [c.n2, c.AUG2], F32, kind="Internal")
    ar_bufs = {}
    for k in range(4):
        ar_bufs[k] = (
            nc.dram_tensor(f"ar_in{k}", [P, 4], F32, kind="Internal"),
            nc.dram_tensor(f"ar_out{k}", [P, 4], F32, kind="Internal"),
        )

    rg = [list(range(c.ncores))]

    with tile.TileContext(nc) as tc:
        glob = tc.alloc_tile_pool(name="glob", bufs=1)
        small = tc.alloc_tile_pool(name="small", bufs=4)
        work = tc.alloc_tile_pool(name="work", bufs=2)
        dps = tc.alloc_tile_pool(name="dps", bufs=2, space="PSUM")
        dmp = tc.alloc_tile_pool(name="dmp", bufs=2, space="PSUM")
        tps = tc.alloc_tile_pool(name="tps", bufs=2, space="PSUM")
        cps = tc.alloc_tile_pool(name="cps", bufs=2, space="PSUM")
        stg = tc.alloc_tile_pool(name="stg", bufs=2)

        # ---------------- setup ----------------
        ident = glob.tile([P, P], F32, tag="ident")
        make_identity(nc, ident)

        iota_i = glob.tile([P, P], I32, tag="iota_i")
        nc.gpsimd.iota(iota_i, pattern=[[1, P]], base=0, channel_multiplier=0)
        iota_f = glob.tile([P, P], F32, tag="iota_f")
        nc.vector.tensor_copy(iota_f, iota_i)

        # weights to SBUF (lhsT layout [128, kc, M]) and param columns
        def load_w(name, kin, mout):
            kc = (kin + P - 1) // P
            stgt = stg.tile([P, kc, mout], F32, tag="stg", name="stg")
            t = glob.tile([P, kc, mout], F32R, tag=f"w_{name}")
            if kin % P != 0:
                nc.vector.memset(stgt, 0.0)
            src = w[name]
            for i in range(kc):
                rows = min(P, kin - i * P)
                nc.sync.dma_start(stgt[:rows, i, :], src[ds(i * P, rows), :])
            nc.vector.tensor_copy(t, stgt)
            return t

        def load_col(name, cdim):
            kc = (cdim + P - 1) // P
            t = glob.tile([P, kc], F32, tag=f"c_{name}")
            if cdim % P:
                nc.vector.memset(t, 0.0)
            nc.sync.dma_start(
                t[: min(P, cdim), : (cdim + P - 1) // P],
                w[name].rearrange("(kc p) -> p kc", p=min(P, cdim)),
            )
            return t

        W = {}
        for name, kin, mout in [
            ("fp3_W1", FP3_IN, FP3_HID), ("fp3_W2", FP3_HID, FP3_OUT),
            ("fp2_W1", FP2_IN, FP2_HID), ("fp2_W2", FP2_HID, FP2_OUT),
            ("fp1_W1", FP1_IN, FP1_HID), ("fp1_W2", FP1_HID, FP1_HID),
            ("fp1_W3", FP1_HID, FP1_OUT),
            ("head_Wa", FP1_OUT, HEAD_HID), ("head_Wb", HEAD_HID, HEAD_HID),
            ("head_Wc", HEAD_HID, HEAD_OUT),
        ]:
            W[name] = load_w(name, kin, mout)
        w1b_f32 = glob.tile([4, FP1_HID], F32, tag="w1b_f32")
        nc.vector.memset(w1b_f32, 0.0)
        nc.sync.dma_start(w1b_f32[0:3, :], w["fp1_W1"][ds(P, 3), :])

        COL = {}
        for name, cdim in [
            ("fp3_g1", FP3_HID), ("fp3_be1", FP3_HID), ("fp3_b2", FP3_OUT),
            ("fp2_g1", FP2_HID), ("fp2_be1", FP2_HID), ("fp2_b2", FP2_OUT),
            ("fp1_g1", FP1_HID), ("fp1_be1", FP1_HID),
            ("fp1_g2", FP1_HID), ("fp1_be2", FP1_HID), ("fp1_b3", FP1_OUT),
            ("head_ba", HEAD_HID), ("head_bb", HEAD_HID), ("head_bc", HEAD_OUT),
        ]:
            COL[name] = load_col(name, cdim)

        # inputs resident in SBUF
        x1T_stg = stg.tile([P, c.n1], F32, tag="stg", name="x1stg")
        nc.sync.dma_start(x1T_stg, x1T[:, :])
        x1T_sb = glob.tile([P, c.n1], F32R, tag="x1T")
        nc.vector.tensor_copy(x1T_sb, x1T_stg)
        x2T_stg = stg.tile([P, 2, c.n2], F32, tag="stg", name="x2stg")
        nc.sync.dma_start(x2T_stg, x2T.rearrange("(kc p) n -> p kc n", p=P))
        x2T_sb = glob.tile([P, 2, c.n2], F32R, tag="x2T")
        nc.vector.tensor_copy(x2T_sb, x2T_stg)

        px0_sb = glob.tile([P, c.T0 + 4, 8], F32, tag="px0")
        nc.vector.memset(px0_sb[:, c.T0 :], 0.0)
        nc.sync.dma_start(px0_sb[:, : c.T0], px0.rearrange("(t p) c -> p t c", p=P))
        pq1_sb = glob.tile([P, c.T1, 4], F32, tag="pq1")
        nc.sync.dma_start(pq1_sb, pq1.rearrange("(t p) c -> p t c", p=P))
        pq2_sb = glob.tile([P, c.T2, 4], F32, tag="pq2")
        nc.sync.dma_start(pq2_sb, pq2.rearrange("(t p) c -> p t c", p=P))
        pos3_sb = glob.tile([c.n3, 3], F32, tag="pos3")
        nc.sync.dma_start(pos3_sb, pos3[:, :])

        # |q|^2 tables (positive and negated)
        def sq_table(src, T, tagp):
            tmp = work.tile([P, T, 3], F32, tag="sqtmp")
            nc.vector.tensor_tensor(tmp, src[:, :, 0:3], src[:, :, 0:3], op=ALU.mult)
            pos_t = glob.tile([P, T], F32, tag=f"{tagp}_sq")
            nc.vector.tensor_reduce(pos_t, tmp, axis=mybir.AxisListType.X, op=ALU.add)
            neg_t = glob.tile([P, T], F32, tag=f"{tagp}_nsq")
            nc.vector.tensor_scalar(neg_t, pos_t, -1.0, None, op0=ALU.mult)
            return pos_t, neg_t

        q0sq, q0nsq = sq_table(px0_sb[:, : c.T0], c.T0, "q0")
        q1sq, q1nsq = sq_table(pq1_sb, c.T1, "q1")
        q2sq, _ = sq_table(pq2_sb, c.T2, "q2")

        # p-tilde tables [4, n] = [2px, 2py, 2pz, |p|^2] (feature-major)
        def pt_table(pm_sb, sq_sb, T, tagp):
            t = glob.tile([4, T * P], F32, tag=f"pt_{tagp}")
            for i in range(T):
                a = work.tile([P, 4], F32, tag="pt_a")
                nc.vector.tensor_scalar(a[:, 0:3], pm_sb[:, i, 0:3], 2.0, None, op0=ALU.mult)
                nc.vector.tensor_copy(a[:, 3:4], sq_sb[:, i : i + 1])
                ps = tps.tile([4, P], F32, tag="t")
                nc.tensor.transpose(ps, a, ident)
                nc.vector.tensor_copy(t[:, ts(i, P)], ps)
            return t

        pt1 = pt_table(pq1_sb, q1sq, c.T1, "pt1")
        pt2 = pt_table(pq2_sb, q2sq, c.T2, "pt2")
        # level-3: [n3, 3] point-major, n3 <= 128
        p3a = work.tile([c.n3, 4], F32, tag="p3a")
        p3t = work.tile([c.n3, 3], F32, tag="p3t")
        nc.vector.tensor_tensor(p3t, pos3_sb, pos3_sb, op=ALU.mult)
        nc.vector.tensor_scalar(p3a[:, 0:3], pos3_sb, 2.0, None, op0=ALU.mult)
        nc.vector.tensor_reduce(p3a[:, 3:4], p3t, axis=mybir.AxisListType.X, op=ALU.add)
        pt3 = glob.tile([4, c.n3], F32, tag="pt3")
        p3ps = tps.tile([4, c.n3], F32, tag="t")
        nc.tensor.transpose(p3ps, p3a, ident[: c.n3, : c.n3])
        nc.vector.tensor_copy(pt3, p3ps)

        # packed buffers (uint16 pairs: [idx | bf16(-d)]) with persistent iota
        def packed_bufs(nbuf, cand, tagp):
            bufs = []
            for i in range(nbuf):
                t = glob.tile([P, 2 * cand], U16, tag=f"{tagp}_{i}")
                pairs = t.rearrange("p (c two) -> p c two", two=2)
                nc.gpsimd.iota(pairs[:, :, 0], pattern=[[1, cand]], base=0,
                               channel_multiplier=0)
                bufs.append(t)
            return bufs

        NPK = 3
        pk1 = packed_bufs(3, c.c1, "pk1")
        pk2 = packed_bufs(2, c.c2, "pk2")

        def qtilde(src_sb, t, dt_out=F32):
            """transpose [128,4] slice -> q~ [4,128] in SBUF"""
            ps = tps.tile([4, P], F32, tag="t")
            nc.tensor.transpose(ps, src_sb[:, t, 0:4], ident)
            qt = work.tile([4, P], dt_out, tag="qt_sb")
            nc.vector.tensor_copy(qt, ps)
            return qt

        # ---------------- KNN screening (fp1 then fp2) ----------------
        def knn_screen(T, tpc, cand, ptab, src_sb, nsq, pkbufs, TB, tagp,
                       tiles=None, offs=None):
            """Returns (offs list, None)."""
            if offs is None:
                offs = []
            off_t = None
            for t in (tiles if tiles is not None else range(T)):
                cl = t // tpc
                tb = t % TB
                qt = qtilde(src_sb, t)
                pk = pkbufs[t % len(pkbufs)]
                pk_bf = pk.bitcast(FP16).rearrange("p (c two) -> p c two", two=2)
                for (o, wd) in _chunks(cand, 512):
                    dp = dmp.tile([P, 512], F32, tag="d")
                    nc.tensor.matmul(dp[:, :wd], qt, ptab[:, cl * cand + o : cl * cand + o + wd],
                                     start=True, stop=True)
                    nc.scalar.activation(pk_bf[:, o : o + wd, 1], dp[:, :wd],
                                         AF.Identity, bias=nsq[:, t : t + 1], scale=1.0)
                top8 = small.tile([P, 8], F32, tag=f"{tagp}_top8")
                nc.vector.max(out=top8, in_=pk.bitcast(F32))
                if tb == 0:
                    off_t = glob.tile([P, TB, KS], U32, tag=f"{tagp}_off{t // TB}")
                idxs = top8.bitcast(U16).rearrange("p (k two) -> p k two", two=2)[:, 0:KS, 0]
                nc.vector.tensor_scalar(off_t[:, tb], idxs, float(cl * cand), None, op0=ALU.add)
                if tb == TB - 1:
                    offs.append(off_t)
            return offs, None

        # fp2 screen first: its offsets gate the fp2 stage which gates fp1.
        offs2, _ = knn_screen(c.T1, c.t1c, c.c2, pt2, pq1_sb, q1nsq, pk2, c.TB2, "k2")

        # ---------------- BN helper ----------------
        ar_idx = [0]

        def bn_apply(z_sb, stats, nchunk, mchunks, gcol, becol, tag):
            """z_sb: [128, mchunks, ...chunks...]; stats [128, mchunks, nchunk, 6].
            Returns (scale, shift) [128, mchunks]."""
            k = ar_idx[0]
            ar_idx[0] += 1
            mv = small.tile([P, mchunks, 2], F32, tag=f"mv_{tag}")
            for m in range(mchunks):
                nc.vector.bn_aggr(out=mv[:, m], in_=stats[:, m])
            add = small.tile([P, mchunks, 2], F32, tag=f"ad_{tag}")
            msq = small.tile([P, mchunks], F32, tag=f"ms_{tag}")
            nc.vector.tensor_copy(add[:, :, 0], mv[:, :, 0])
            nc.vector.tensor_tensor(msq, mv[:, :, 0], mv[:, :, 0], op=ALU.mult)
            nc.vector.tensor_tensor(add[:, :, 1], mv[:, :, 1], msq, op=ALU.add)
            g = small.tile([P, mchunks, 2], F32, tag=f"g_{tag}")
            if do_ar:
                arin, arout = ar_bufs[k]
                nc.sync.dma_start(arin[:, 0 : 2 * mchunks],
                                  add.rearrange("p m two -> p (m two)"))
                nc.gpsimd.collective_compute(
                    "AllReduce", ALU.add, replica_groups=rg,
                    ins=[arin.ap()], outs=[arout.ap()],
                )
                nc.sync.dma_start(g.rearrange("p m two -> p (m two)"),
                                  arout[:, 0 : 2 * mchunks])
                nc.vector.tensor_scalar(g, g, 1.0 / c.ncores, None, op0=ALU.mult)
            else:
                nc.vector.tensor_copy(g, add)
            mean = small.tile([P, mchunks], F32, tag=f"mn_{tag}")
            var = small.tile([P, mchunks], F32, tag=f"vr_{tag}")
            nc.vector.tensor_copy(mean, g[:, :, 0])
            nc.vector.tensor_tensor(msq, mean, mean, op=ALU.mult)
            nc.vector.tensor_tensor(var, g[:, :, 1], msq, op=ALU.subtract)
            nc.vector.tensor_scalar(var, var, EPS_BN, None, op0=ALU.add)
            rin = small.tile([P, mchunks], F32, tag=f"ri_{tag}")
            nc.vector.reciprocal(rin, var)
            sinv = small.tile([P, mchunks], F32, tag=f"si_{tag}")
            nc.scalar.activation(sinv, rin, AF.Sqrt)
            sc = small.tile([P, mchunks], F32, tag=f"sc_{tag}")
            sh = small.tile([P, mchunks], F32, tag=f"sh_{tag}")
            nc.vector.tensor_tensor(sc, gcol[:, 0:mchunks], sinv, op=ALU.mult)
            nc.vector.tensor_tensor(sh, mean, sc, op=ALU.mult)
            nc.vector.tensor_tensor(sh, becol[:, 0:mchunks], sh, op=ALU.subtract)
            return sc, sh

        # ---------------- fp3 stage ----------------
        stg.release()
        fp3 = tc.alloc_tile_pool(name="fp3", bufs=1)
        x3p = []
        for cl in range(c.cpc):
            t = fp3.tile([P, C3F], F32, tag=f"x3p{cl}", name=f"x3p{cl}")
            nc.vector.memset(t, 0.0)
            nc.sync.dma_start(t[: c.c3, :], x3[ds(cl * c.c3, c.c3), :])
            x3p.append(t)
        h3T = fp3.tile([P, C3F // P, c.n2], F32R, tag="h3T")
        for t in range(c.T2):
            cl = t // c.t2c
            qt = qtilde(pq2_sb, t, dt_out=F32)
            dp = cps.tile([P, P], F32, tag="c")
            nc.tensor.matmul(dp[:, : c.c3], qt, pt3[:, ds(cl * c.c3, c.c3)],
                             start=True, stop=True)
            dn = work.tile([P, c.c3], F32, tag="dn3")
            nc.vector.tensor_copy(dn, dp[:, : c.c3])
            top8 = small.tile([P, 8], F32, tag="t83")
            nc.vector.max(out=top8, in_=dn)
            idx8 = small.tile([P, 8], U32, tag="i83")
            nc.vector.max_index(idx8, top8, dn)
            idxf = small.tile([P, 1], F32, tag="if3")
            nc.vector.tensor_copy(idxf, idx8[:, 0:1])
            oh = work.tile([P, P], F32, tag="oh3")
            nc.vector.tensor_scalar(oh, iota_f, idxf, None, op0=ALU.is_equal)
            ohps = tps.tile([P, P], F32, tag="t")
            nc.tensor.transpose(ohps, oh, ident)
            ohT = work.tile([P, P], F32, tag="ohT")
            nc.vector.tensor_copy(ohT, ohps)
            for j in range(C3F // P):
                hp = cps.tile([P, P], F32, tag="c")
                nc.tensor.matmul(hp, x3p[cl][:, ts(j, P)], ohT, start=True, stop=True)
                nc.vector.tensor_copy(h3T[:, j, ts(t, P)], hp)

        # fp3 MLP layer 1 (1280 -> 256), BN, relu; layer 2 (256 -> 256)
        n2ch = _chunks(c.n2, 512)
        z1f3 = fp3.tile([P, 2, c.n2], F32, tag="z1f3")
        st3 = fp3.tile([P, 2, len(n2ch), 6], F32, tag="st3")
        KC3 = C3F // P
        for m in range(2):
            for (o, wd) in n2ch:
                zp = dps.tile([P, 512], F32, tag="z")
                for kc in range(KC3 + 2):
                    rhs = h3T[:, kc, o : o + wd] if kc < KC3 else x2T_sb[:, kc - KC3, o : o + wd]
                    nc.tensor.matmul(zp[:, :wd], W["fp3_W1"][:, kc, ts(m, P)], rhs,
                                     start=(kc == 0), stop=(kc == KC3 + 1))
                nc.vector.tensor_copy(z1f3[:, m, o : o + wd], zp[:, :wd])
            for i, (o, wd) in enumerate(n2ch):
                nc.vector.bn_stats(out=st3[:, m, i], in_=z1f3[:, m, o : o + wd])
        sc3, sh3 = bn_apply(z1f3, st3, len(n2ch), 2, COL["fp3_g1"], COL["fp3_be1"], "bn3")
        h2f = fp3.tile([P, 2, c.n2], F32, tag="h2f")
        z1n3 = fp3.tile([P, 2, c.n2], F32R, tag="z1n3")
        for m in range(2):
            nc.scalar.activation(z1n3[:, m], z1f3[:, m], AF.Relu,
                                 bias=sh3[:, m : m + 1], scale=sc3[:, m : m + 1])
        for m in range(2):
            for (o, wd) in n2ch:
                zp = dps.tile([P, 512], F32, tag="z")
                for kc in range(2):
                    nc.tensor.matmul(zp[:, :wd], W["fp3_W2"][:, kc, ts(m, P)],
                                     z1n3[:, kc, o : o + wd],
                                     start=(kc == 0), stop=(kc == 1))
                nc.vector.tensor_scalar(h2f[:, m, o : o + wd], zp[:, :wd],
                                        COL["fp3_b2"][:, m : m + 1], None, op0=ALU.add)

        # h2aug table: [h2 (256) | pos2 (3) | p2^2] rows, to DRAM
        aug2 = fp3.tile([P, c.T2, c.AUG2], F32, tag="aug2")
        for t in range(c.T2):
            for m in range(2):
                hps = tps.tile([P, P], F32, tag="t")
                nc.tensor.transpose(hps, h2f[:, m, ts(t, P)], ident)
                nc.vector.tensor_copy(aug2[:, t, ts(m, P)], hps)
        nc.vector.tensor_copy(aug2[:, :, 2 * P : 2 * P + 3], pq2_sb[:, :, 0:3])
        nc.vector.tensor_copy(aug2[:, :, 2 * P + 3 : 2 * P + 4],
                              q2sq.unsqueeze(-1))
        nc.sync.dma_start(h2aug_d.rearrange("(t p) c -> p t c", p=P), aug2)
        fp3.release()

        # ---------------- gather + refine + combine helper ----------------
        def gather_combine_batch(b, off_t, aug_d, augw, featw, pm_sb, qsq_t, TB, consume, tagp, pool):
            """one batch: indirect-gather KS rows/query, refine weights in
            fp32, diag-matmul combine -> consume(t, psum_tiles_by_mchunk)."""
            mch = featw // P
            if True:
                gt = pool.tile([P, TB, KS, augw], F32, tag=f"{tagp}_g", name=f"{tagp}_g", bufs=2)
                for tb in range(TB):
                    for k in range(KS):
                        nc.gpsimd.indirect_dma_start(
                            out=gt[:, tb, k], out_offset=None, in_=aug_d[:, :],
                            in_offset=IndirectOffsetOnAxis(
                                ap=off_t[:, tb, k : k + 1], axis=0),
                        )
                tsl = slice(b * TB, b * TB + TB)
                # exact d for survivors: d = q2 + p2 - 2 q.p
                gpos = gt[:, :, :, featw : featw + 3]
                qpos = pm_sb[:, tsl, 0:3].unsqueeze(2).to_broadcast(
                    [P, TB, KS, 3])
                prod = pool.tile([P, TB, KS, 3], F32, tag=f"{tagp}_pr", name=f"{tagp}_pr", bufs=2)
                nc.vector.tensor_tensor(prod, gpos, qpos, op=ALU.mult)
                dot = pool.tile([P, TB, KS], F32, tag=f"{tagp}_dot", name=f"{tagp}_dot", bufs=2)
                nc.vector.tensor_reduce(dot, prod, axis=mybir.AxisListType.X, op=ALU.add)
                d4 = pool.tile([P, TB, KS], F32, tag=f"{tagp}_d4", name=f"{tagp}_d4", bufs=2)
                nc.vector.tensor_scalar(d4, dot, -2.0, None, op0=ALU.mult)
                nc.vector.tensor_tensor(d4, d4, gt[:, :, :, featw + 3], op=ALU.add)
                qsq_b = qsq_t[:, tsl].unsqueeze(-1).to_broadcast([P, TB, KS])
                nc.vector.tensor_tensor(d4, d4, qsq_b, op=ALU.add)
                # w = 1/max(d, 1e-16); drop smallest w (largest d); normalize
                nc.vector.tensor_scalar(d4, d4, 1e-16, None, op0=ALU.max)
                wv = pool.tile([P, TB, KS], F32, tag=f"{tagp}_w", name=f"{tagp}_w", bufs=2)
                nc.vector.reciprocal(wv, d4)
                wmin = pool.tile([P, TB], F32, tag=f"{tagp}_wm", name=f"{tagp}_wm", bufs=2)
                nc.vector.tensor_reduce(wmin, wv, axis=mybir.AxisListType.X, op=ALU.min)
                wsum = pool.tile([P, TB], F32, tag=f"{tagp}_ws", name=f"{tagp}_ws", bufs=2)
                nc.vector.tensor_reduce(wsum, wv, axis=mybir.AxisListType.X, op=ALU.add)
                nc.vector.tensor_tensor(wsum, wsum, wmin, op=ALU.subtract)
                rs = pool.tile([P, TB], F32, tag=f"{tagp}_rs", name=f"{tagp}_rs", bufs=2)
                nc.vector.reciprocal(rs, wsum)
                wmin_b = wmin.unsqueeze(-1).to_broadcast([P, TB, KS])
                keep = pool.tile([P, TB, KS], F32, tag=f"{tagp}_k", name=f"{tagp}_k", bufs=2)
                nc.vector.tensor_tensor(keep, wv, wmin_b, op=ALU.is_equal)
                nc.vector.tensor_scalar(keep, keep, -1.0, 1.0, op0=ALU.mult, op1=ALU.add)
                nc.vector.tensor_tensor(wv, wv, keep, op=ALU.mult)
                rs_b = rs.unsqueeze(-1).to_broadcast([P, TB, KS])
                nc.vector.tensor_tensor(wv, wv, rs_b, op=ALU.mult)
                # diag weight mats and combine (per tile)
                for tb in range(TB):
                    t = b * TB + tb
                    dg = pool.tile([P, KS, P], F32, tag=f"{tagp}_dg", name=f"{tagp}_dg", bufs=2)
                    wv_b = wv[:, tb].unsqueeze(-1).to_broadcast([P, KS, P])
                    ident_b = ident.unsqueeze(1).to_broadcast([P, KS, P])
                    nc.vector.tensor_tensor(dg, ident_b, wv_b, op=ALU.mult)
                    pst = []
                    for m in range(mch):
                        cp = cps.tile([P, P], F32, tag="c")
                        for k in range(KS):
                            nc.tensor.matmul(cp, gt[:, tb, k, ts(m, P)], dg[:, k],
                                             start=(k == 0), stop=(k == KS - 1))
                        pst.append(cp)
                    consume(t, pst)

        # ---------------- fp2 stage ----------------
        fp2 = tc.alloc_tile_pool(name="fp2", bufs=1)
        h2iT = fp2.tile([P, 2, c.n1], F32R, tag="h2iT")

        def consume2(t, pst):
            for m, cp in enumerate(pst):
                nc.vector.tensor_copy(h2iT[:, m, ts(t, P)], cp)

        for b, off_t in enumerate(offs2):
            gather_combine_batch(b, off_t, h2aug_d, c.AUG2, FP3_OUT, pq1_sb, q1sq,
                                 c.TB2, consume2, "g2", fp2)

        n1ch = _chunks(c.n1, 512)
        z1f2 = fp2.tile([P, 2, c.n1], F32, tag="z1f2")
        st2s = fp2.tile([P, 2, len(n1ch), 6], F32, tag="st2")
        for m in range(2):
            for (o, wd) in n1ch:
                zp = dps.tile([P, 512], F32, tag="z")
                for kc in range(3):
                    rhs = h2iT[:, kc, o : o + wd] if kc < 2 else x1T_sb[:, o : o + wd]
                    nc.tensor.matmul(zp[:, :wd], W["fp2_W1"][:, kc, ts(m, P)], rhs,
                                     start=(kc == 0), stop=(kc == 2))
                nc.vector.tensor_copy(z1f2[:, m, o : o + wd], zp[:, :wd])
            for i, (o, wd) in enumerate(n1ch):
                nc.vector.bn_stats(out=st2s[:, m, i], in_=z1f2[:, m, o : o + wd])
        sc2, sh2 = bn_apply(z1f2, st2s, len(n1ch), 2, COL["fp2_g1"], COL["fp2_be1"], "bn2")
        h1f = fp2.tile([P, c.n1], F32, tag="h1f")
        for (o, wd) in n1ch:
            z1n = work.tile([P, 2, 512], F32R, tag="z1n2")
            for m in range(2):
                nc.scalar.activation(z1n[:, m, :wd], z1f2[:, m, o : o + wd], AF.Relu,
                                     bias=sh2[:, m : m + 1], scale=sc2[:, m : m + 1])
            zp = dps.tile([P, 512], F32, tag="z")
            for kc in range(2):
                nc.tensor.matmul(zp[:, :wd], W["fp2_W2"][:, kc, :], z1n[:, kc, :wd],
                                 start=(kc == 0), stop=(kc == 1))
            nc.vector.tensor_scalar(h1f[:, o : o + wd], zp[:, :wd],
                                    COL["fp2_b2"][:, 0:1], None, op0=ALU.add)

        aug1 = fp2.tile([P, c.T1, c.AUG1], F32, tag="aug1")
        for t in range(c.T1):
            hps = tps.tile([P, P], F32, tag="t")
            nc.tensor.transpose(hps, h1f[:, ts(t, P)], ident)
            nc.vector.tensor_copy(aug1[:, t, 0:P], hps)
        nc.vector.tensor_copy(aug1[:, :, P : P + 3], pq1_sb[:, :, 0:3])
        nc.vector.tensor_copy(aug1[:, :, P + 3 : P + 4], q1sq.unsqueeze(-1))
        nc.sync.dma_start(h1aug_d.rearrange("(t p) c -> p t c", p=P), aug1)
        fp2.release()

        # ---------------- fp1 stage + head ----------------
        fp1 = tc.alloc_tile_pool(name="fp1", bufs=1)
        n0ch = _chunks(c.n0, 512)
        z1sb = fp1.tile([P, len(n0ch), 512], F32, tag="z1sb")
        st1 = fp1.tile([P, 1, len(n0ch), 6], F32, tag="st1")
        rhs_pool = tc.alloc_tile_pool(name="rhs1", bufs=2)
        state = {"rhs": None, "x0T": None}

        def consume1(t, pst):
            b = t // c.TB1
            tb = t % c.TB1
            if tb == 0:
                state["rhs"] = rhs_pool.tile([P, c.TB1 * P], F32R, tag="r1", name="r1")
                state["x0T"] = rhs_pool.tile([4, c.TB1 * P], F32, tag="x0T", name="x0T")
            nc.vector.tensor_copy(state["rhs"][:, ts(tb, P)], pst[0])
            xps = tps.tile([4, P], F32, tag="t")
            nc.tensor.transpose(xps, px0_sb[:, t, 4:8], ident)
            nc.vector.tensor_copy(state["x0T"][:, ts(tb, P)], xps)
            if tb == c.TB1 - 1:
                wd = c.TB1 * P
                zp = dps.tile([P, 512], F32, tag="z")
                nc.tensor.matmul(zp[:, :wd], W["fp1_W1"][:, 0, :], state["rhs"],
                                 start=True, stop=False)
                nc.tensor.matmul(zp[:, :wd], w1b_f32[0:3, :], state["x0T"][:3],
                                 start=False, stop=True)
                nc.vector.tensor_copy(z1sb[:, b, :wd], zp[:, :wd])
                nc.vector.bn_stats(out=st1[:, 0, b], in_=z1sb[:, b, :wd])

        nb1 = c.T0 // c.TB1
        for b in range(nb1):
            obs, qx = knn_screen(c.T0, c.t0c, c.c1, pt1, px0_sb, q0nsq, pk1, c.TB1,
                                 "k1", tiles=range(b * c.TB1, (b + 1) * c.TB1))
            gather_combine_batch(b, obs[0], h1aug_d, c.AUG1, FP2_OUT, px0_sb, q0sq,
                                 c.TB1, consume1, "g1", fp1)

        sc1a, sh1a = bn_apply(z1sb, st1, len(n0ch), 1, COL["fp1_g1"], COL["fp1_be1"], "bn1a")
        z2sb = fp1.tile([P, len(n0ch), 512], F32, tag="z2sb")
        st1b = fp1.tile([P, 1, len(n0ch), 6], F32, tag="st1b")
        for i, (o, wd) in enumerate(n0ch):
            z1n = work.tile([P, 512], F32R, tag="z1n1")
            nc.scalar.activation(z1n[:, :wd], z1sb[:, i, :wd], AF.Relu,
                                 bias=sh1a[:, 0:1], scale=sc1a[:, 0:1])
            zp = dps.tile([P, 512], F32, tag="z")
            nc.tensor.matmul(zp[:, :wd], W["fp1_W2"][:, 0, :], z1n[:, :wd],
                             start=True, stop=True)
            nc.vector.tensor_copy(z2sb[:, i, :wd], zp[:, :wd])
            nc.vector.bn_stats(out=st1b[:, 0, i], in_=z2sb[:, i, :wd])
        sc1b, sh1b = bn_apply(z2sb, st1b, len(n0ch), 1, COL["fp1_g2"], COL["fp1_be2"], "bn1b")

        for i, (o, wd) in enumerate(n0ch):
            z2n = work.tile([P, 512], F32R, tag="z2n1")
            nc.scalar.activation(z2n[:, :wd], z2sb[:, i, :wd], AF.Relu,
                                 bias=sh1b[:, 0:1], scale=sc1b[:, 0:1])
            zp = dps.tile([P, 512], F32, tag="z")
            nc.tensor.matmul(zp[:, :wd], W["fp1_W3"][:, 0, :], z2n[:, :wd],
                             start=True, stop=True)
            z3c = work.tile([P, 512], F32R, tag="z3c")
            nc.vector.tensor_scalar(z3c[:, :wd], zp[:, :wd], COL["fp1_b3"][:, 0:1],
                                    None, op0=ALU.add)
            hap = dps.tile([P, 512], F32, tag="z")
            nc.tensor.matmul(hap[:, :wd], W["head_Wa"][:, 0, :], z3c[:, :wd],
                             start=True, stop=True)
            hac = work.tile([P, 512], F32R, tag="hac")
            nc.scalar.activation(hac[:, :wd], hap[:, :wd], AF.Relu,
                                 bias=COL["head_ba"][:, 0:1], scale=1.0)
            hbp = dps.tile([P, 512], F32, tag="z")
            nc.tensor.matmul(hbp[:, :wd], W["head_Wb"][:, 0, :], hac[:, :wd],
                             start=True, stop=True)
            hbc = work.tile([P, 512], F32R, tag="hbc")
            nc.scalar.activation(hbc[:, :wd], hbp[:, :wd], AF.Relu,
                                 bias=COL["head_bb"][:, 0:1], scale=1.0)
            op = cps.tile([HEAD_OUT, 512], F32, tag="c")
            nc.tensor.matmul(op[:, :wd], W["head_Wc"][:, 0, :], hbc[:, :wd],
                             start=True, stop=True)
            oc = work.tile([HEAD_OUT, 512], F32, tag="oc")
            nc.vector.tensor_scalar(oc[:, :wd], op[:, :wd], COL["head_bc"][:HEAD_OUT, 0:1],
                                    None, op0=ALU.add)
            nc.sync.dma_start(outT[:, o : o + wd], oc[:, :wd])

        rhs_pool.release()
        fp1.release()
        cps.release()
        tps.release()
        dmp.release()
        dps.release()
        work.release()
        small.release()
        glob.release()

    return nc


# ============================ host side ============================

def shard_inputs(inputs, cfg: Cfg):
    """Split full inputs into per-core input maps (layout only, no math)."""
    c = cfg
    npt = c.npt

    def g(name):
        v = inputs[name]
        return np.ascontiguousarray(np.asarray(v), dtype=np.float32)

    x0, pos0 = g("x0"), g("pos0")
    x1, pos1 = g("x1"), g("pos1")
    x2, pos2 = g("x2"), g("pos2")
    x3, pos3 = g("x3"), g("pos3")
    fp3p = [np.asarray(t, np.float32) for t in inputs["fp3_params"]]
    fp2p = [np.asarray(t, np.float32) for t in inputs["fp2_params"]]
    fp1p = [np.asarray(t, np.float32) for t in inputs["fp1_params"]]
    headp = [np.asarray(t, np.float32) for t in inputs["head_params"]]

    wmap = {
        "fp3_W1": fp3p[0], "fp3_g1": fp3p[2], "fp3_be1": fp3p[3],
        "fp3_W2": fp3p[4], "fp3_b2": fp3p[5],
        "fp2_W1": fp2p[0], "fp2_g1": fp2p[2], "fp2_be1": fp2p[3],
        "fp2_W2": fp2p[4], "fp2_b2": fp2p[5],
        "fp1_W1": fp1p[0], "fp1_g1": fp1p[2], "fp1_be1": fp1p[3],
        "fp1_W2": fp1p[4], "fp1_g2": fp1p[6], "fp1_be2": fp1p[7],
        "fp1_W3": fp1p[8], "fp1_b3": fp1p[9],
        "head_Wa": headp[0], "head_ba": headp[1],
        "head_Wb": headp[2], "head_bb": headp[3],
        "head_Wc": headp[4], "head_bc": headp[5],
    }
    wmap = {k: np.ascontiguousarray(v) for k, v in wmap.items()}

    maps = []
    for core in range(c.ncores):
        s0 = slice(core * c.n0, (core + 1) * c.n0)
        s1 = slice(core * c.n1, (core + 1) * c.n1)
        s2 = slice(core * c.n2, (core + 1) * c.n2)
        s3 = slice(core * c.n3, (core + 1) * c.n3)
        px0 = np.concatenate(
            [pos0[s0], np.full((c.n0, 1), -1.0, np.float32), x0[s0],
             np.zeros((c.n0, 1), np.float32)], axis=1)
        pq1 = np.concatenate([pos1[s1], np.full((c.n1, 1), -1.0, np.float32)], axis=1)
        pq2 = np.concatenate([pos2[s2], np.full((c.n2, 1), -1.0, np.float32)], axis=1)
        m = {
            "px0": np.ascontiguousarray(px0),
            "pq1": np.ascontiguousarray(pq1),
            "pq2": np.ascontiguousarray(pq2),
            "pos3": np.ascontiguousarray(pos3[s3]),
            "x1T": np.ascontiguousarray(x1[s1].T),
            "x2T": np.ascontiguousarray(x2[s2].T),
            "x3": np.ascontiguousarray(x3[s3]),
        }
        m.update(wmap)
        maps.append(m)
    return maps


_CACHE = {}


def _get_compiled(cfg_key):
    if cfg_key in _CACHE:
        return _CACHE[cfg_key]
    cfg = Cfg()
    nc = bacc.Bacc("TRN2", target_bir_lowering=False, debug=False,
                   num_devices=cfg.ncores)
    build(nc, cfg)
    nc.compile()
    _CACHE[cfg_key] = (nc, cfg)
    return nc, cfg


def kernel(**inputs):
    from concourse.bass_utils import run_bass_kernel_spmd

    nc, cfg = _get_compiled("full")
    in_maps = shard_inputs(inputs, cfg)
    trace = bool(int(os.environ.get("KERNEL_TRACE", "0")))
    res = run_bass_kernel_spmd(nc, in_maps, core_ids=list(range(cfg.ncores)),
                               trace=trace)
    if trace and res.exec_time_ns is not None:
        print(f"HW exec time: {res.exec_time_ns} ns")
    outs = [r["outT"] for r in res.results]
    full = np.concatenate([np.ascontiguousarray(o.T) for o in outs], axis=0)
    kernel.last_results = res
    return full


# revision 29
# speedup vs baseline: 1.1449x; 1.1449x over previous
"""Trainium2 Bass kernel for nn_Decoder_8950711845590 (PointNet++-style decoder).

Self-contained: builds, compiles and runs an 8-core SPMD Bass/Tile kernel.
Sharding: data-parallel over point clouds (2 clouds per core), MLP weights
replicated; BatchNorm batch-statistics are combined across cores with a tiny
AllReduce per BN layer.

Algorithm highlights (per core):
- KNN distances via PE matmul with K=4 ("[qx,qy,qz,-1] . [2px,2py,2pz,|p|^2]").
- top-k via one DVE max8 pass over values packed as fp32 = [bf16(-d) | idx]
  (value-sorted with index tie-break, index extracted from the low 16 bits).
- k=3 stages screen top-4 in bf16, then re-compute exact fp32 distances for
  the 4 survivors from gathered candidate coordinates and drop the farthest.
- feature gather through indirect DMA from DRAM row tables [feat | pos | p^2].
- weighted combine with PE matmuls against per-query diagonal weight matrices.
- feature-major MLPs (channels on partitions) so BN stats are free-dim reduces.
"""

import os
import numpy as np

import concourse.bass as bass
import concourse.mybir as mybir
import concourse.tile as tile
from concourse import bacc
from concourse.bass import ts, ds, IndirectOffsetOnAxis
from concourse.masks import make_identity

AF = mybir.ActivationFunctionType
ALU = mybir.AluOpType
DT = mybir.dt
F32 = DT.float32
F32R = DT.float32r
BF16 = DT.bfloat16
FP16 = DT.float16
U16 = DT.uint16
U32 = DT.uint32
I32 = DT.int32

P = 128
EPS_BN = 1e-5

# problem constants (full config)
B_FULL = 16
NPT_FULL = (4096, 1024, 256, 64)
NCORES_FULL = 8
CPC = 2  # clouds per core

# feature dims (fixed by the MLP weights)
C3F = 1024   # x3 feature dim
FP3_IN, FP3_HID, FP3_OUT = 1280, 256, 256
FP2_IN, FP2_HID, FP2_OUT = 384, 256, 128
FP1_IN, FP1_HID, FP1_OUT = 131, 128, 128
HEAD_HID, HEAD_OUT = 128, 3

KS = int(os.environ.get('KERNEL_KS', '4'))  # survivors per query (3 = no refine-drop)


def _chunks(n, c):
    out = []
    o = 0
    while o < n:
        w = min(c, n - o)
        out.append((o, w))
        o += w
    return out


class Cfg:
    def __init__(self, npt=NPT_FULL, cpc=CPC, ncores=NCORES_FULL):
        self.npt = npt
        self.cpc = cpc
        self.ncores = ncores
        self.n0, self.n1, self.n2, self.n3 = [cpc * n for n in npt]
        self.c1, self.c2, self.c3 = npt[1], npt[2], npt[3]
        assert self.n0 % P == 0 and npt[0] % P == 0
        assert npt[1] % P == 0 and npt[2] % P == 0
        assert self.c3 <= P and self.n3 <= P
        self.T0 = self.n0 // P          # fp1 query tiles
        self.T1 = self.n1 // P          # fp2 query tiles (and h1aug tiles)
        self.T2 = self.n2 // P          # fp3 query tiles (and h2aug tiles)
        self.t0c = npt[0] // P          # tiles per cloud
        self.t1c = npt[1] // P
        self.t2c = npt[2] // P
        self.TB1 = min(4, self.T0)      # gather batch (fp1), = 512-pt N chunk
        self.TB2 = min(2, self.T1)
        self.AUG1 = FP2_OUT + 4         # 132
        self.AUG2 = FP3_OUT + 4         # 260


def build(nc, cfg: Cfg):
    c = cfg
    do_ar = c.ncores > 1

    # ---------------- DRAM I/O ----------------
    def din(name, shape):
        return nc.dram_tensor(name, list(shape), F32, kind="ExternalInput")

    px0 = din("px0", (c.n0, 8))          # [pos0 | -1 | x0 | 0]
    pq1 = din("pq1", (c.n1, 4))          # [pos1 | -1]
    pq2 = din("pq2", (c.n2, 4))          # [pos2 | -1]
    pos3 = din("pos3", (c.n3, 3))
    x1T = din("x1T", (FP2_OUT, c.n1))
    x2T = din("x2T", (FP3_OUT, c.n2))
    x3 = din("x3", (c.n3, C3F))
    w = {}
    for name, shape in [
        ("fp3_W1", (FP3_IN, FP3_HID)), ("fp3_g1", (FP3_HID,)), ("fp3_be1", (FP3_HID,)),
        ("fp3_W2", (FP3_HID, FP3_OUT)), ("fp3_b2", (FP3_OUT,)),
        ("fp2_W1", (FP2_IN, FP2_HID)), ("fp2_g1", (FP2_HID,)), ("fp2_be1", (FP2_HID,)),
        ("fp2_W2", (FP2_HID, FP2_OUT)), ("fp2_b2", (FP2_OUT,)),
        ("fp1_W1", (FP1_IN, FP1_HID)), ("fp1_g1", (FP1_HID,)), ("fp1_be1", (FP1_HID,)),
        ("fp1_W2", (FP1_HID, FP1_HID)), ("fp1_g2", (FP1_HID,)), ("fp1_be2", (FP1_HID,)),
        ("fp1_W3", (FP1_HID, FP1_OUT)), ("fp1_b3", (FP1_OUT,)),
        ("head_Wa", (FP1_OUT, HEAD_HID)), ("head_ba", (HEAD_HID,)),
        ("head_Wb", (HEAD_HID, HEAD_HID)), ("head_bb", (HEAD_HID,)),
        ("head_Wc", (HEAD_HID, HEAD_OUT)), ("head_bc", (HEAD_OUT,)),
    ]:
        w[name] = din(name, shape)
    outT = nc.dram_tensor("outT", [HEAD_OUT, c.n0], F32, kind="ExternalOutput")

    # DRAM scratch
    h1aug_d = nc.dram_tensor("h1aug", [c.n1, c.AUG1], F32, kind="Internal")
    h2aug_d = nc.dram_tensor("h2aug", [c.n2, c.AUG2], F32, kind="Internal")
    ar_bufs = {}
    for k in range(4):
        ar_bufs[k] = (
            nc.dram_tensor(f"ar_in{k}", [P, 4], F32, kind="Internal"),
            nc.dram_tensor(f"ar_out{k}", [P, 4], F32, kind="Internal"),
        )

    rg = [list(range(c.ncores))]

    with tile.TileContext(nc) as tc:
        glob = tc.alloc_tile_pool(name="glob", bufs=1)
        small = tc.alloc_tile_pool(name="small", bufs=4)
        work = tc.alloc_tile_pool(name="work", bufs=2)
        dps = tc.alloc_tile_pool(name="dps", bufs=2, space="PSUM")
        dmp = tc.alloc_tile_pool(name="dmp", bufs=2, space="PSUM")
        tps = tc.alloc_tile_pool(name="tps", bufs=2, space="PSUM")
        cps = tc.alloc_tile_pool(name="cps", bufs=2, space="PSUM")
        stg = tc.alloc_tile_pool(name="stg", bufs=2)

        # ---------------- setup ----------------
        ident = glob.tile([P, P], F32, tag="ident")
        make_identity(nc, ident)

        iota_i = glob.tile([P, P], I32, tag="iota_i")
        nc.gpsimd.iota(iota_i, pattern=[[1, P]], base=0, channel_multiplier=0)
        iota_f = glob.tile([P, P], F32, tag="iota_f")
        nc.vector.tensor_copy(iota_f, iota_i)

        # weights to SBUF (lhsT layout [128, kc, M]) and param columns
        def load_w(name, kin, mout):
            kc = (kin + P - 1) // P
            stgt = stg.tile([P, kc, mout], F32, tag="stg", name="stg")
            t = glob.tile([P, kc, mout], F32R, tag=f"w_{name}")
            if kin % P != 0:
                nc.vector.memset(stgt, 0.0)
            src = w[name]
            for i in range(kc):
                rows = min(P, kin - i * P)
                nc.sync.dma_start(stgt[:rows, i, :], src[ds(i * P, rows), :])
            nc.vector.tensor_copy(t, stgt)
            return t

        def load_col(name, cdim):
            kc = (cdim + P - 1) // P
            t = glob.tile([P, kc], F32, tag=f"c_{name}")
            if cdim % P:
                nc.vector.memset(t, 0.0)
            nc.sync.dma_start(
                t[: min(P, cdim), : (cdim + P - 1) // P],
                w[name].rearrange("(kc p) -> p kc", p=min(P, cdim)),
            )
            return t

        W = {}
        for name, kin, mout in [
            ("fp3_W1", FP3_IN, FP3_HID), ("fp3_W2", FP3_HID, FP3_OUT),
            ("fp2_W1", FP2_IN, FP2_HID), ("fp2_W2", FP2_HID, FP2_OUT),
            ("fp1_W1", FP1_IN, FP1_HID), ("fp1_W2", FP1_HID, FP1_HID),
            ("fp1_W3", FP1_HID, FP1_OUT),
            ("head_Wa", FP1_OUT, HEAD_HID), ("head_Wb", HEAD_HID, HEAD_HID),
            ("head_Wc", HEAD_HID, HEAD_OUT),
        ]:
            W[name] = load_w(name, kin, mout)
        w1b_f32 = glob.tile([4, FP1_HID], F32, tag="w1b_f32")
        nc.vector.memset(w1b_f32, 0.0)
        nc.sync.dma_start(w1b_f32[0:3, :], w["fp1_W1"][ds(P, 3), :])

        COL = {}
        for name, cdim in [
            ("fp3_g1", FP3_HID), ("fp3_be1", FP3_HID), ("fp3_b2", FP3_OUT),
            ("fp2_g1", FP2_HID), ("fp2_be1", FP2_HID), ("fp2_b2", FP2_OUT),
            ("fp1_g1", FP1_HID), ("fp1_be1", FP1_HID),
            ("fp1_g2", FP1_HID), ("fp1_be2", FP1_HID), ("fp1_b3", FP1_OUT),
            ("head_ba", HEAD_HID), ("head_bb", HEAD_HID), ("head_bc", HEAD_OUT),
        ]:
            COL[name] = load_col(name, cdim)

        # inputs resident in SBUF
        x1T_stg = stg.tile([P, c.n1], F32, tag="stg", name="x1stg")
        nc.sync.dma_start(x1T_stg, x1T[:, :])
        x1T_sb = glob.tile([P, c.n1], F32R, tag="x1T")
        nc.vector.tensor_copy(x1T_sb, x1T_stg)
        x2T_stg = stg.tile([P, 2, c.n2], F32, tag="stg", name="x2stg")
        nc.sync.dma_start(x2T_stg, x2T.rearrange("(kc p) n -> p kc n", p=P))
        x2T_sb = glob.tile([P, 2, c.n2], F32R, tag="x2T")
        nc.vector.tensor_copy(x2T_sb, x2T_stg)

        px0_sb = glob.tile([P, c.T0 + 4, 8], F32, tag="px0")
        nc.vector.memset(px0_sb[:, c.T0 :], 0.0)
        nc.sync.dma_start(px0_sb[:, : c.T0], px0.rearrange("(t p) c -> p t c", p=P))
        pq1_sb = glob.tile([P, c.T1, 4], F32, tag="pq1")
        nc.sync.dma_start(pq1_sb, pq1.rearrange("(t p) c -> p t c", p=P))
        pq2_sb = glob.tile([P, c.T2, 4], F32, tag="pq2")
        nc.sync.dma_start(pq2_sb, pq2.rearrange("(t p) c -> p t c", p=P))
        pos3_sb = glob.tile([c.n3, 3], F32, tag="pos3")
        nc.sync.dma_start(pos3_sb, pos3[:, :])

        # |q|^2 tables (positive and negated)
        def sq_table(src, T, tagp):
            tmp = work.tile([P, T, 3], F32, tag="sqtmp")
            nc.vector.tensor_tensor(tmp, src[:, :, 0:3], src[:, :, 0:3], op=ALU.mult)
            pos_t = glob.tile([P, T], F32, tag=f"{tagp}_sq")
            nc.vector.tensor_reduce(pos_t, tmp, axis=mybir.AxisListType.X, op=ALU.add)
            neg_t = glob.tile([P, T], F32, tag=f"{tagp}_nsq")
            nc.vector.tensor_scalar(neg_t, pos_t, -1.0, None, op0=ALU.mult)
            return pos_t, neg_t

        q0sq, q0nsq = sq_table(px0_sb[:, : c.T0], c.T0, "q0")
        q1sq, q1nsq = sq_table(pq1_sb, c.T1, "q1")
        q2sq, _ = sq_table(pq2_sb, c.T2, "q2")

        # p-tilde tables [4, n] = [2px, 2py, 2pz, |p|^2] (feature-major)
        def pt_table(pm_sb, sq_sb, T, tagp):
            t = glob.tile([4, T * P], F32, tag=f"pt_{tagp}")
            for i in range(T):
                a = work.tile([P, 4], F32, tag="pt_a")
                nc.vector.tensor_scalar(a[:, 0:3], pm_sb[:, i, 0:3], 2.0, None, op0=ALU.mult)
                nc.vector.tensor_copy(a[:, 3:4], sq_sb[:, i : i + 1])
                ps = tps.tile([4, P], F32, tag="t")
                nc.tensor.transpose(ps, a, ident)
                nc.vector.tensor_copy(t[:, ts(i, P)], ps)
            return t

        pt1 = pt_table(pq1_sb, q1sq, c.T1, "pt1")
        pt2 = pt_table(pq2_sb, q2sq, c.T2, "pt2")
        # level-3: [n3, 3] point-major, n3 <= 128
        p3a = work.tile([c.n3, 4], F32, tag="p3a")
        p3t = work.tile([c.n3, 3], F32, tag="p3t")
        nc.vector.tensor_tensor(p3t, pos3_sb, pos3_sb, op=ALU.mult)
        nc.vector.tensor_scalar(p3a[:, 0:3], pos3_sb, 2.0, None, op0=ALU.mult)
        nc.vector.tensor_reduce(p3a[:, 3:4], p3t, axis=mybir.AxisListType.X, op=ALU.add)
        pt3 = glob.tile([4, c.n3], F32, tag="pt3")
        p3ps = tps.tile([4, c.n3], F32, tag="t")
        nc.tensor.transpose(p3ps, p3a, ident[: c.n3, : c.n3])
        nc.vector.tensor_copy(pt3, p3ps)

        # packed buffers (uint16 pairs: [idx | bf16(-d)]) with persistent iota
        def packed_bufs(nbuf, cand, tagp):
            bufs = []
            for i in range(nbuf):
                t = glob.tile([P, 2 * cand], U16, tag=f"{tagp}_{i}")
                pairs = t.rearrange("p (c two) -> p c two", two=2)
                nc.gpsimd.iota(pairs[:, :, 0], pattern=[[1, cand]], base=0,
                               channel_multiplier=0)
                bufs.append(t)
            return bufs

        NPK = 3
        pk1 = packed_bufs(3, c.c1, "pk1")
        pk2 = packed_bufs(2, c.c2, "pk2")

        def qtilde(src_sb, t, dt_out=F32):
            """transpose [128,4] slice -> q~ [4,128] in SBUF"""
            ps = tps.tile([4, P], F32, tag="t")
            nc.tensor.transpose(ps, src_sb[:, t, 0:4], ident)
            qt = work.tile([4, P], dt_out, tag="qt_sb")
            nc.vector.tensor_copy(qt, ps)
            return qt

        # ---------------- KNN screening (fp1 then fp2) ----------------
        def knn_screen(T, tpc, cand, ptab, src_sb, nsq, pkbufs, TB, tagp,
                       tiles=None, offs=None):
            """Returns (offs list, None)."""
            if offs is None:
                offs = []
            off_t = None
            for t in (tiles if tiles is not None else range(T)):
                cl = t // tpc
                tb = t % TB
                qt = qtilde(src_sb, t)
                pk = pkbufs[t % len(pkbufs)]
                pk_bf = pk.bitcast(FP16).rearrange("p (c two) -> p c two", two=2)
                for (o, wd) in _chunks(cand, 512):
                    dp = dmp.tile([P, 512], F32, tag="d")
                    nc.tensor.matmul(dp[:, :wd], qt, ptab[:, cl * cand + o : cl * cand + o + wd],
                                     start=True, stop=True)
                    nc.scalar.activation(pk_bf[:, o : o + wd, 1], dp[:, :wd],
                                         AF.Identity, bias=nsq[:, t : t + 1], scale=1.0)
                top8 = small.tile([P, 8], F32, tag=f"{tagp}_top8")
                nc.vector.max(out=top8, in_=pk.bitcast(F32))
                if tb == 0:
                    off_t = glob.tile([P, TB, KS], U32, tag=f"{tagp}_off{t // TB}")
                idxs = top8.bitcast(U16).rearrange("p (k two) -> p k two", two=2)[:, 0:KS, 0]
                nc.vector.tensor_scalar(off_t[:, tb], idxs, float(cl * cand), None, op0=ALU.add)
                if tb == TB - 1:
                    offs.append(off_t)
            return offs, None

        # fp2 screen first: its offsets gate the fp2 stage which gates fp1.
        offs2, _ = knn_screen(c.T1, c.t1c, c.c2, pt2, pq1_sb, q1nsq, pk2, c.TB2, "k2")

        # ---------------- BN helper ----------------
        ar_idx = [0]

        def bn_apply(z_sb, stats, nchunk, mchunks, gcol, becol, tag):
            """z_sb: [128, mchunks, ...chunks...]; stats [128, mchunks, nchunk, 6].
            Returns (scale, shift) [128, mchunks]."""
            k = ar_idx[0]
            ar_idx[0] += 1
            mv = small.tile([P, mchunks, 2], F32, tag=f"mv_{tag}")
            for m in range(mchunks):
                nc.vector.bn_aggr(out=mv[:, m], in_=stats[:, m])
            add = small.tile([P, mchunks, 2], F32, tag=f"ad_{tag}")
            msq = small.tile([P, mchunks], F32, tag=f"ms_{tag}")
            nc.vector.tensor_copy(add[:, :, 0], mv[:, :, 0])
            nc.vector.tensor_tensor(msq, mv[:, :, 0], mv[:, :, 0], op=ALU.mult)
            nc.vector.tensor_tensor(add[:, :, 1], mv[:, :, 1], msq, op=ALU.add)
            g = small.tile([P, mchunks, 2], F32, tag=f"g_{tag}")
            if do_ar:
                arin, arout = ar_bufs[k]
                nc.sync.dma_start(arin[:, 0 : 2 * mchunks],
                                  add.rearrange("p m two -> p (m two)"))
                nc.gpsimd.collective_compute(
                    "AllReduce", ALU.add, replica_groups=rg,
                    ins=[arin.ap()], outs=[arout.ap()],
                )
                nc.sync.dma_start(g.rearrange("p m two -> p (m two)"),
                                  arout[:, 0 : 2 * mchunks])
                nc.vector.tensor_scalar(g, g, 1.0 / c.ncores, None, op0=ALU.mult)
            else:
                nc.vector.tensor_copy(g, add)
            mean = small.tile([P, mchunks], F32, tag=f"mn_{tag}")
            var = small.tile([P, mchunks], F32, tag=f"vr_{tag}")
            nc.vector.tensor_copy(mean, g[:, :, 0])
            nc.vector.tensor_tensor(msq, mean, mean, op=ALU.mult)
            nc.vector.tensor_tensor(var, g[:, :, 1], msq, op=ALU.subtract)
            nc.vector.tensor_scalar(var, var, EPS_BN, None, op0=ALU.add)
            rin = small.tile([P, mchunks], F32, tag=f"ri_{tag}")
            nc.vector.reciprocal(rin, var)
            sinv = small.tile([P, mchunks], F32, tag=f"si_{tag}")
            nc.scalar.activation(sinv, rin, AF.Sqrt)
            sc = small.tile([P, mchunks], F32, tag=f"sc_{tag}")
            sh = small.tile([P, mchunks], F32, tag=f"sh_{tag}")
            nc.vector.tensor_tensor(sc, gcol[:, 0:mchunks], sinv, op=ALU.mult)
            nc.vector.tensor_tensor(sh, mean, sc, op=ALU.mult)
            nc.vector.tensor_tensor(sh, becol[:, 0:mchunks], sh, op=ALU.subtract)
            return sc, sh

        # ---------------- fp3 stage ----------------
        stg.release()
        fp3 = tc.alloc_tile_pool(name="fp3", bufs=1)
        x3p = []
        for cl in range(c.cpc):
            t = fp3.tile([P, C3F], F32, tag=f"x3p{cl}", name=f"x3p{cl}")
            nc.vector.memset(t, 0.0)
            nc.sync.dma_start(t[: c.c3, :], x3[ds(cl * c.c3, c.c3), :])
            x3p.append(t)
        h3T = fp3.tile([P, C3F // P, c.n2], F32R, tag="h3T")
        for t in range(c.T2):
            cl = t // c.t2c
            qt = qtilde(pq2_sb, t, dt_out=F32)
            dp = cps.tile([P, P], F32, tag="c")
            nc.tensor.matmul(dp[:, : c.c3], qt, pt3[:, ds(cl * c.c3, c.c3)],
                             start=True, stop=True)
            dn = work.tile([P, c.c3], F32, tag="dn3")
            nc.vector.tensor_copy(dn, dp[:, : c.c3])
            top8 = small.tile([P, 8], F32, tag="t83")
            nc.vector.max(out=top8, in_=dn)
            idx8 = small.tile([P, 8], U32, tag="i83")
            nc.vector.max_index(idx8, top8, dn)
            idxf = small.tile([P, 1], F32, tag="if3")
            nc.vector.tensor_copy(idxf, idx8[:, 0:1])
            oh = work.tile([P, P], F32, tag="oh3")
            nc.vector.tensor_scalar(oh, iota_f, idxf, None, op0=ALU.is_equal)
            ohps = tps.tile([P, P], F32, tag="t")
            nc.tensor.transpose(ohps, oh, ident)
            ohT = work.tile([P, P], F32, tag="ohT")
            nc.vector.tensor_copy(ohT, ohps)
            for j in range(C3F // P):
                hp = cps.tile([P, P], F32, tag="c")
                nc.tensor.matmul(hp, x3p[cl][:, ts(j, P)], ohT, start=True, stop=True)
                nc.vector.tensor_copy(h3T[:, j, ts(t, P)], hp)

        # fp3 MLP layer 1 (1280 -> 256), BN, relu; layer 2 (256 -> 256)
        n2ch = _chunks(c.n2, 512)
        z1f3 = fp3.tile([P, 2, c.n2], F32, tag="z1f3")
        st3 = fp3.tile([P, 2, len(n2ch), 6], F32, tag="st3")
        KC3 = C3F // P
        for m in range(2):
            for (o, wd) in n2ch:
                zp = dps.tile([P, 512], F32, tag="z")
                for kc in range(KC3 + 2):
                    rhs = h3T[:, kc, o : o + wd] if kc < KC3 else x2T_sb[:, kc - KC3, o : o + wd]
                    nc.tensor.matmul(zp[:, :wd], W["fp3_W1"][:, kc, ts(m, P)], rhs,
                                     start=(kc == 0), stop=(kc == KC3 + 1))
                nc.vector.tensor_copy(z1f3[:, m, o : o + wd], zp[:, :wd])
            for i, (o, wd) in enumerate(n2ch):
                nc.vector.bn_stats(out=st3[:, m, i], in_=z1f3[:, m, o : o + wd])
        sc3, sh3 = bn_apply(z1f3, st3, len(n2ch), 2, COL["fp3_g1"], COL["fp3_be1"], "bn3")
        h2f = fp3.tile([P, 2, c.n2], F32, tag="h2f")
        z1n3 = fp3.tile([P, 2, c.n2], F32R, tag="z1n3")
        for m in range(2):
            nc.scalar.activation(z1n3[:, m], z1f3[:, m], AF.Relu,
                                 bias=sh3[:, m : m + 1], scale=sc3[:, m : m + 1])
        for m in range(2):
            for (o, wd) in n2ch:
                zp = dps.tile([P, 512], F32, tag="z")
                for kc in range(2):
                    nc.tensor.matmul(zp[:, :wd], W["fp3_W2"][:, kc, ts(m, P)],
                                     z1n3[:, kc, o : o + wd],
                                     start=(kc == 0), stop=(kc == 1))
                nc.vector.tensor_scalar(h2f[:, m, o : o + wd], zp[:, :wd],
                                        COL["fp3_b2"][:, m : m + 1], None, op0=ALU.add)

        # h2aug table: [h2 (256) | pos2 (3) | p2^2] rows, to DRAM
        aug2 = fp3.tile([P, c.T2, c.AUG2], F32, tag="aug2")
        for t in range(c.T2):
            for m in range(2):
                hps = tps.tile([P, P], F32, tag="t")
                nc.tensor.transpose(hps, h2f[:, m, ts(t, P)], ident)
                nc.vector.tensor_copy(aug2[:, t, ts(m, P)], hps)
        nc.vector.tensor_copy(aug2[:, :, 2 * P : 2 * P + 3], pq2_sb[:, :, 0:3])
        nc.vector.tensor_copy(aug2[:, :, 2 * P + 3 : 2 * P + 4],
                              q2sq.unsqueeze(-1))
        nc.sync.dma_start(h2aug_d.rearrange("(t p) c -> p t c", p=P), aug2)
        fp3.release()

        # ---------------- gather + refine + combine helper ----------------
        def gather_combine_batch(b, off_t, aug_d, augw, featw, pm_sb, qsq_t, TB, consume, tagp, pool):
            """one batch: indirect-gather KS rows/query, refine weights in
            fp32, diag-matmul combine -> consume(t, psum_tiles_by_mchunk)."""
            mch = featw // P
            if True:
                gt = pool.tile([P, TB, KS, augw], F32, tag=f"{tagp}_g", name=f"{tagp}_g", bufs=2)
                for tb in range(TB):
                    for k in range(KS):
                        nc.gpsimd.indirect_dma_start(
                            out=gt[:, tb, k], out_offset=None, in_=aug_d[:, :],
                            in_offset=IndirectOffsetOnAxis(
                                ap=off_t[:, tb, k : k + 1], axis=0),
                        )
                tsl = slice(b * TB, b * TB + TB)
                # exact d for survivors: d = q2 + p2 - 2 q.p
                gpos = gt[:, :, :, featw : featw + 3]
                qpos = pm_sb[:, tsl, 0:3].unsqueeze(2).to_broadcast(
                    [P, TB, KS, 3])
                prod = pool.tile([P, TB, KS, 3], F32, tag=f"{tagp}_pr", name=f"{tagp}_pr", bufs=2)
                nc.vector.tensor_tensor(prod, gpos, qpos, op=ALU.mult)
                dot = pool.tile([P, TB, KS], F32, tag=f"{tagp}_dot", name=f"{tagp}_dot", bufs=2)
                nc.vector.tensor_reduce(dot, prod, axis=mybir.AxisListType.X, op=ALU.add)
                d4 = pool.tile([P, TB, KS], F32, tag=f"{tagp}_d4", name=f"{tagp}_d4", bufs=2)
                nc.vector.tensor_scalar(d4, dot, -2.0, None, op0=ALU.mult)
                nc.vector.tensor_tensor(d4, d4, gt[:, :, :, featw + 3], op=ALU.add)
                qsq_b = qsq_t[:, tsl].unsqueeze(-1).to_broadcast([P, TB, KS])
                nc.vector.tensor_tensor(d4, d4, qsq_b, op=ALU.add)
                # w = 1/max(d, 1e-16); (KS=4: drop smallest w); normalize
                nc.vector.tensor_scalar(d4, d4, 1e-16, None, op0=ALU.max)
                wv = pool.tile([P, TB, KS], F32, tag=f"{tagp}_w", name=f"{tagp}_w", bufs=2)
                nc.vector.reciprocal(wv, d4)
                wsum = pool.tile([P, TB], F32, tag=f"{tagp}_ws", name=f"{tagp}_ws", bufs=2)
                nc.vector.tensor_reduce(wsum, wv, axis=mybir.AxisListType.X, op=ALU.add)
                if KS > 3:
                    wmin = pool.tile([P, TB], F32, tag=f"{tagp}_wm", name=f"{tagp}_wm", bufs=2)
                    nc.vector.tensor_reduce(wmin, wv, axis=mybir.AxisListType.X, op=ALU.min)
                    nc.vector.tensor_tensor(wsum, wsum, wmin, op=ALU.subtract)
                    wmin_b = wmin.unsqueeze(-1).to_broadcast([P, TB, KS])
                    keep = pool.tile([P, TB, KS], F32, tag=f"{tagp}_k", name=f"{tagp}_k", bufs=2)
                    nc.vector.tensor_tensor(keep, wv, wmin_b, op=ALU.is_equal)
                    nc.vector.tensor_scalar(keep, keep, -1.0, 1.0, op0=ALU.mult, op1=ALU.add)
                    nc.vector.tensor_tensor(wv, wv, keep, op=ALU.mult)
                rs = pool.tile([P, TB], F32, tag=f"{tagp}_rs", name=f"{tagp}_rs", bufs=2)
                nc.vector.reciprocal(rs, wsum)
                rs_b = rs.unsqueeze(-1).to_broadcast([P, TB, KS])
                nc.vector.tensor_tensor(wv, wv, rs_b, op=ALU.mult)
                # diag weight mats and combine (per tile)
                for tb in range(TB):
                    t = b * TB + tb
                    dg = pool.tile([P, KS, P], F32, tag=f"{tagp}_dg", name=f"{tagp}_dg", bufs=2)
                    wv_b = wv[:, tb].unsqueeze(-1).to_broadcast([P, KS, P])
                    ident_b = ident.unsqueeze(1).to_broadcast([P, KS, P])
                    nc.vector.tensor_tensor(dg, ident_b, wv_b, op=ALU.mult)
                    pst = []
                    for m in range(mch):
                        cp = cps.tile([P, P], F32, tag="c")
                        for k in range(KS):
                            nc.tensor.matmul(cp, gt[:, tb, k, ts(m, P)], dg[:, k],
                                             start=(k == 0), stop=(k == KS - 1))
                        pst.append(cp)
                    consume(t, pst)

        # ---------------- fp2 stage ----------------
        fp2 = tc.alloc_tile_pool(name="fp2", bufs=1)
        h2iT = fp2.tile([P, 2, c.n1], F32R, tag="h2iT")

        def consume2(t, pst):
            for m, cp in enumerate(pst):
                nc.vector.tensor_copy(h2iT[:, m, ts(t, P)], cp)

        for b, off_t in enumerate(offs2):
            gather_combine_batch(b, off_t, h2aug_d, c.AUG2, FP3_OUT, pq1_sb, q1sq,
                                 c.TB2, consume2, "g2", fp2)

        n1ch = _chunks(c.n1, 512)
        z1f2 = fp2.tile([P, 2, c.n1], F32, tag="z1f2")
        st2s = fp2.tile([P, 2, len(n1ch), 6], F32, tag="st2")
        for m in range(2):
            for (o, wd) in n1ch:
                zp = dps.tile([P, 512], F32, tag="z")
                for kc in range(3):
                    rhs = h2iT[:, kc, o : o + wd] if kc < 2 else x1T_sb[:, o : o + wd]
                    nc.tensor.matmul(zp[:, :wd], W["fp2_W1"][:, kc, ts(m, P)], rhs,
                                     start=(kc == 0), stop=(kc == 2))
                nc.vector.tensor_copy(z1f2[:, m, o : o + wd], zp[:, :wd])
            for i, (o, wd) in enumerate(n1ch):
                nc.vector.bn_stats(out=st2s[:, m, i], in_=z1f2[:, m, o : o + wd])
        sc2, sh2 = bn_apply(z1f2, st2s, len(n1ch), 2, COL["fp2_g1"], COL["fp2_be1"], "bn2")
        h1f = fp2.tile([P, c.n1], F32, tag="h1f")
        for (o, wd) in n1ch:
            z1n = work.tile([P, 2, 512], F32R, tag="z1n2")
            for m in range(2):
                nc.scalar.activation(z1n[:, m, :wd], z1f2[:, m, o : o + wd], AF.Relu,
                                     bias=sh2[:, m : m + 1], scale=sc2[:, m : m + 1])
            zp = dps.tile([P, 512], F32, tag="z")
            for kc in range(2):
                nc.tensor.matmul(zp[:, :wd], W["fp2_W2"][:, kc, :], z1n[:, kc, :wd],
                                 start=(kc == 0), stop=(kc == 1))
            nc.vector.tensor_scalar(h1f[:, o : o + wd], zp[:, :wd],
                                    COL["fp2_b2"][:, 0:1], None, op0=ALU.add)

        aug1 = fp2.tile([P, c.T1, c.AUG1], F32, tag="aug1")
        for t in range(c.T1):
            hps = tps.tile([P, P], F32, tag="t")
            nc.tensor.transpose(hps, h1f[:, ts(t, P)], ident)
            nc.vector.tensor_copy(aug1[:, t, 0:P], hps)
        nc.vector.tensor_copy(aug1[:, :, P : P + 3], pq1_sb[:, :, 0:3])
        nc.vector.tensor_copy(aug1[:, :, P + 3 : P + 4], q1sq.unsqueeze(-1))
        nc.sync.dma_start(h1aug_d.rearrange("(t p) c -> p t c", p=P), aug1)
        fp2.release()

        # ---------------- fp1 stage + head ----------------
        fp1 = tc.alloc_tile_pool(name="fp1", bufs=1)
        n0ch = _chunks(c.n0, 512)
        z1sb = fp1.tile([P, len(n0ch), 512], F32, tag="z1sb")
        st1 = fp1.tile([P, 1, len(n0ch), 6], F32, tag="st1")
        rhs_pool = tc.alloc_tile_pool(name="rhs1", bufs=2)
        state = {"rhs": None, "x0T": None}

        def consume1(t, pst):
            b = t // c.TB1
            tb = t % c.TB1
            if tb == 0:
                state["rhs"] = rhs_pool.tile([P, c.TB1 * P], F32R, tag="r1", name="r1")
                state["x0T"] = rhs_pool.tile([4, c.TB1 * P], F32, tag="x0T", name="x0T")
            nc.vector.tensor_copy(state["rhs"][:, ts(tb, P)], pst[0])
            xps = tps.tile([4, P], F32, tag="t")
            nc.tensor.transpose(xps, px0_sb[:, t, 4:8], ident)
            nc.vector.tensor_copy(state["x0T"][:, ts(tb, P)], xps)
            if tb == c.TB1 - 1:
                wd = c.TB1 * P
                zp = dps.tile([P, 512], F32, tag="z")
                nc.tensor.matmul(zp[:, :wd], W["fp1_W1"][:, 0, :], state["rhs"],
                                 start=True, stop=False)
                nc.tensor.matmul(zp[:, :wd], w1b_f32[0:3, :], state["x0T"][:3],
                                 start=False, stop=True)
                nc.vector.tensor_copy(z1sb[:, b, :wd], zp[:, :wd])
                nc.vector.bn_stats(out=st1[:, 0, b], in_=z1sb[:, b, :wd])

        nb1 = c.T0 // c.TB1
        for b in range(nb1):
            obs, qx = knn_screen(c.T0, c.t0c, c.c1, pt1, px0_sb, q0nsq, pk1, c.TB1,
                                 "k1", tiles=range(b * c.TB1, (b + 1) * c.TB1))
            gather_combine_batch(b, obs[0], h1aug_d, c.AUG1, FP2_OUT, px0_sb, q0sq,
                                 c.TB1, consume1, "g1", fp1)

        sc1a, sh1a = bn_apply(z1sb, st1, len(n0ch), 1, COL["fp1_g1"], COL["fp1_be1"], "bn1a")
        z2sb = fp1.tile([P, len(n0ch), 512], F32, tag="z2sb")
        st1b = fp1.tile([P, 1, len(n0ch), 6], F32, tag="st1b")
        for i, (o, wd) in enumerate(n0ch):
            z1n = work.tile([P, 512], F32R, tag="z1n1")
            nc.scalar.activation(z1n[:, :wd], z1sb[:, i, :wd], AF.Relu,
                                 bias=sh1a[:, 0:1], scale=sc1a[:, 0:1])
            zp = dps.tile([P, 512], F32, tag="z")
            nc.tensor.matmul(zp[:, :wd], W["fp1_W2"][:, 0, :], z1n[:, :wd],
                             start=True, stop=True)
            nc.vector.tensor_copy(z2sb[:, i, :wd], zp[:, :wd])
            nc.vector.bn_stats(out=st1b[:, 0, i], in_=z2sb[:, i, :wd])
        sc1b, sh1b = bn_apply(z2sb, st1b, len(n0ch), 1, COL["fp1_g2"], COL["fp1_be2"], "bn1b")

        for i, (o, wd) in enumerate(n0ch):
            z2n = work.tile([P, 512], F32R, tag="z2n1")
            nc.scalar.activation(z2n[:, :wd], z2sb[:, i, :wd], AF.Relu,
                                 bias=sh1b[:, 0:1], scale=sc1b[:, 0:1])
            zp = dps.tile([P, 512], F32, tag="z")
            nc.tensor.matmul(zp[:, :wd], W["fp1_W3"][:, 0, :], z2n[:, :wd],
                             start=True, stop=True)
            z3c = work.tile([P, 512], F32R, tag="z3c")
            nc.vector.tensor_scalar(z3c[:, :wd], zp[:, :wd], COL["fp1_b3"][:, 0:1],
                                    None, op0=ALU.add)
            hap = dps.tile([P, 512], F32, tag="z")
            nc.tensor.matmul(hap[:, :wd], W["head_Wa"][:, 0, :], z3c[:, :wd],
                             start=True, stop=True)
            hac = work.tile([P, 512], F32R, tag="hac")
            nc.scalar.activation(hac[:, :wd], hap[:, :wd], AF.Relu,
                                 bias=COL["head_ba"][:, 0:1], scale=1.0)
            hbp = dps.tile([P, 512], F32, tag="z")
            nc.tensor.matmul(hbp[:, :wd], W["head_Wb"][:, 0, :], hac[:, :wd],
                             start=True, stop=True)
            hbc = work.tile([P, 512], F32R, tag="hbc")
            nc.scalar.activation(hbc[:, :wd], hbp[:, :wd], AF.Relu,
                                 bias=COL["head_bb"][:, 0:1], scale=1.0)
            op = cps.tile([HEAD_OUT, 512], F32, tag="c")
            nc.tensor.matmul(op[:, :wd], W["head_Wc"][:, 0, :], hbc[:, :wd],
                             start=True, stop=True)
            oc = work.tile([HEAD_OUT, 512], F32, tag="oc")
            nc.vector.tensor_scalar(oc[:, :wd], op[:, :wd], COL["head_bc"][:HEAD_OUT, 0:1],
                                    None, op0=ALU.add)
            nc.sync.dma_start(outT[:, o : o + wd], oc[:, :wd])

        rhs_pool.release()
        fp1.release()
        cps.release()
        tps.release()
        dmp.release()
        dps.release()
        work.release()
        small.release()
        glob.release()

    return nc


# ============================ host side ============================

def shard_inputs(inputs, cfg: Cfg):
    """Split full inputs into per-core input maps (layout only, no math)."""
    c = cfg
    npt = c.npt

    def g(name):
        v = inputs[name]
        return np.ascontiguousarray(np.asarray(v), dtype=np.float32)

    x0, pos0 = g("x0"), g("pos0")
    x1, pos1 = g("x1"), g("pos1")
    x2, pos2 = g("x2"), g("pos2")
    x3, pos3 = g("x3"), g("pos3")
    fp3p = [np.asarray(t, np.float32) for t in inputs["fp3_params"]]
    fp2p = [np.asarray(t, np.float32) for t in inputs["fp2_params"]]
    fp1p = [np.asarray(t, np.float32) for t in inputs["fp1_params"]]
    headp = [np.asarray(t, np.float32) for t in inputs["head_params"]]

    wmap = {
        "fp3_W1": fp3p[0], "fp3_g1": fp3p[2], "fp3_be1": fp3p[3],
        "fp3_W2": fp3p[4], "fp3_b2": fp3p[5],
        "fp2_W1": fp2p[0], "fp2_g1": fp2p[2], "fp2_be1": fp2p[3],
        "fp2_W2": fp2p[4], "fp2_b2": fp2p[5],
        "fp1_W1": fp1p[0], "fp1_g1": fp1p[2], "fp1_be1": fp1p[3],
        "fp1_W2": fp1p[4], "fp1_g2": fp1p[6], "fp1_be2": fp1p[7],
        "fp1_W3": fp1p[8], "fp1_b3": fp1p[9],
        "head_Wa": headp[0], "head_ba": headp[1],
        "head_Wb": headp[2], "head_bb": headp[3],
        "head_Wc": headp[4], "head_bc": headp[5],
    }
    wmap = {k: np.ascontiguousarray(v) for k, v in wmap.items()}

    maps = []
    for core in range(c.ncores):
        s0 = slice(core * c.n0, (core + 1) * c.n0)
        s1 = slice(core * c.n1, (core + 1) * c.n1)
        s2 = slice(core * c.n2, (core + 1) * c.n2)
        s3 = slice(core * c.n3, (core + 1) * c.n3)
        px0 = np.concatenate(
            [pos0[s0], np.full((c.n0, 1), -1.0, np.float32), x0[s0],
             np.zeros((c.n0, 1), np.float32)], axis=1)
        pq1 = np.concatenate([pos1[s1], np.full((c.n1, 1), -1.0, np.float32)], axis=1)
        pq2 = np.concatenate([pos2[s2], np.full((c.n2, 1), -1.0, np.float32)], axis=1)
        m = {
            "px0": np.ascontiguousarray(px0),
            "pq1": np.ascontiguousarray(pq1),
            "pq2": np.ascontiguousarray(pq2),
            "pos3": np.ascontiguousarray(pos3[s3]),
            "x1T": np.ascontiguousarray(x1[s1].T),
            "x2T": np.ascontiguousarray(x2[s2].T),
            "x3": np.ascontiguousarray(x3[s3]),
        }
        m.update(wmap)
        maps.append(m)
    return maps


_CACHE = {}


def _get_compiled(cfg_key):
    if cfg_key in _CACHE:
        return _CACHE[cfg_key]
    cfg = Cfg()
    nc = bacc.Bacc("TRN2", target_bir_lowering=False, debug=False,
                   num_devices=cfg.ncores)
    build(nc, cfg)
    nc.compile()
    _CACHE[cfg_key] = (nc, cfg)
    return nc, cfg


def kernel(**inputs):
    from concourse.bass_utils import run_bass_kernel_spmd

    nc, cfg = _get_compiled("full")
    in_maps = shard_inputs(inputs, cfg)
    trace = bool(int(os.environ.get("KERNEL_TRACE", "0")))
    res = run_bass_kernel_spmd(nc, in_maps, core_ids=list(range(cfg.ncores)),
                               trace=trace)
    if trace and res.exec_time_ns is not None:
        print(f"HW exec time: {res.exec_time_ns} ns")
    outs = [r["outT"] for r in res.results]
    full = np.concatenate([np.ascontiguousarray(o.T) for o in outs], axis=0)
    kernel.last_results = res
    return full
